# revision 1
# baseline (speedup 1.0000x reference)
"""CPC loss kernel for Trainium2, batch-sharded across 8 NeuronCores.

Shapes (hardcoded per problem spec):
  z, c: [2048, 64, 128] f32;  mask, neg_map: [128, 64] int;  W: [128, 128] f32
  ln_weight/ln_bias: [128] f32.  Output: scalar f32.

Per-core plan (Bc = 8 batch elements), bf16 data path:
  - Host packs per-core bf16 row tables zf/cf [SEQ*Bc, 128], int32 meta
    (interleaved pos/neg gather indices + keep multiplier), wpack = [W'^T|I].
  - One fused indirect DMA gathers all 2*L*Bc z rows (pos/neg interleaved per
    batch), a second gathers the L*Bc c rows — only ~0.77MB read per core.
  - LN stats via wide free-dim reduces; rstd = exp(-0.5*ln(var+eps)) on ACT
    (tables preloaded early by dummy ops); normalize via chunked wide ops so
    early batches start while later chunks still run.
  - All transposes via the DMA transpose xbar (2-byte dtype) on the Sync
    engine - the PE only runs the two bf16 GEMM stages.
  - exp on ACT with accum_out producing the softmax denominator; numerator =
    diag of exp(MT) via identity mask + reduce.
  - Device outputs num/den [128, 2*Bc]; host does log(num/den + 1e-3) and
    the mean in float64.

ln_weight folds into W on the host; ln_bias cancels in the softmax.  The keep
multiplier folds into rstd.  No max-subtraction needed: |logits| < ~70.
"""

import numpy as np

SEQ, B, L, ZD, CD = 2048, 64, 128, 128, 128
NCORES = 8
BC = B // NCORES  # 8
NSEG = 2 * BC  # 16 LN segments per core (interleaved pos/neg)
LN_EPS = 1e-5

_cached = None


def _build_program():
    import concourse.bacc as bacc
    import concourse.tile as tile
    from concourse import bass, mybir

    f32 = mybir.dt.float32
    bf16 = mybir.dt.bfloat16
    i32 = mybir.dt.int32
    AF = mybir.ActivationFunctionType
    ALU = mybir.AluOpType
    AX = mybir.AxisListType

    nc = bacc.Bacc(
        "TRN2",
        target_bir_lowering=False,
        debug=False,
        enable_asserts=True,
        num_devices=NCORES,
    )

    zf_d = nc.dram_tensor("zf", [SEQ * BC, ZD], bf16, kind="ExternalInput")
    cf_d = nc.dram_tensor("cf", [SEQ * BC, CD], bf16, kind="ExternalInput")
    # meta: [:,0:16] interleaved pos/neg idx, [:,16:24] keep, [:,24:32] pos idx
    meta_d = nc.dram_tensor("meta", [L, 4 * BC], i32, kind="ExternalInput")
    # wpack: [:,0:128] = W'^T, [:,128:256] = identity (bf16)
    wpack_d = nc.dram_tensor("wpack", [128, 256], bf16, kind="ExternalInput")
    out_d = nc.dram_tensor("out", [128, NSEG], f32, kind="ExternalOutput")

    with tile.TileContext(nc) as tc:
        with (
            tc.tile_pool(name="singles", bufs=1) as singles,
            tc.tile_pool(name="scratch", bufs=3) as scratch,
            tc.tile_pool(name="pwide", bufs=2, space="PSUM") as pwide,
            tc.tile_pool(name="pzt", bufs=3, space="PSUM") as ppzt,
            tc.tile_pool(name="pmt", bufs=3, space="PSUM") as ppmt,
        ):
            # ---- small inputs first: gathers depend only on meta ----
            meta_sb = singles.tile([L, 4 * BC], i32)
            nc.sync.dma_start(meta_sb[:], meta_d.ap())
            wpack_sb = singles.tile([128, 256], bf16)
            nc.sync.dma_start(wpack_sb[:], wpack_d.ap())
            wt_sb = wpack_sb[:, 0:128]
            ident_b = wpack_sb[:, 128:256]

            zidx = meta_sb[:, 0:NSEG]
            pos_idx = meta_sb[:, 3 * BC : 4 * BC]
            keep = meta_sb[:, NSEG : NSEG + BC].bitcast(f32)

            # ---- gathers: fused z (pos/neg interleaved), then c ----
            zall = singles.tile([128, NSEG * ZD], bf16)
            call = singles.tile([128, BC * CD], bf16)
            for h in range(2):
                nc.gpsimd.indirect_dma_start(
                    out=zall[:, h * BC * ZD : (h + 1) * BC * ZD],
                    out_offset=None,
                    in_=zf_d.ap(),
                    in_offset=bass.IndirectOffsetOnAxis(
                        ap=zidx[:, h * BC : (h + 1) * BC], axis=0
                    ),
                )
            nc.gpsimd.indirect_dma_start(
                out=call[:],
                out_offset=None,
                in_=cf_d.ap(),
                in_offset=bass.IndirectOffsetOnAxis(ap=pos_idx, axis=0),
            )

            # ---- constants / ACT table preloads during the gather window ----
            identf = singles.tile([128, 128], f32)
            nc.scalar.copy(identf[:], wpack_sb[:, 128:256])
            junk = singles.tile([1, 1], f32)
            nc.vector.memset(junk[:], 1.0)
            nc.scalar.activation(junk[:], junk[:], AF.Square)

            # ---- c-side: PE-transpose ct per batch, batched E = W' @ ct^T --
            ctT_sb = singles.tile([CD, BC * L], bf16)
            for g in range(2):
                pct = pwide.tile([128, 512], bf16, tag="pw")
                for k in range(4):
                    b = g * 4 + k
                    nc.tensor.transpose(
                        out=pct[:, k * 128 : (k + 1) * 128],
                        in_=call[:, b * CD : (b + 1) * CD],
                        identity=ident_b,
                    )
                nc.scalar.copy(ctT_sb[:, g * 512 : (g + 1) * 512], pct[:])
            e_sb = singles.tile([ZD, BC * L], bf16)
            for g in range(2):
                pe = pwide.tile([128, 512], f32, tag="pw")
                nc.tensor.matmul(
                    out=pe[:],
                    lhsT=wt_sb,
                    rhs=ctT_sb[:, g * 512 : (g + 1) * 512],
                    start=True,
                    stop=True,
                )
                nc.scalar.copy(e_sb[:, g * 512 : (g + 1) * 512], pe[:])

            # ---- layernorm, split by halves so batches 0-3 unblock early ----
            z3 = zall[:].rearrange("p (s d) -> p s d", d=ZD)
            s1 = singles.tile([128, NSEG], f32)
            sq = singles.tile([128, NSEG * ZD], f32)
            sq3 = sq[:].rearrange("p (s d) -> p s d", d=ZD)
            s2 = singles.tile([128, NSEG], f32)
            mu = singles.tile([128, NSEG], f32)
            musq = singles.tile([128, NSEG], f32)
            var = singles.tile([128, NSEG], f32)
            vv = singles.tile([128, NSEG], f32)
            y = singles.tile([128, NSEG], f32)
            t1 = singles.tile([128, NSEG], f32)
            zln = singles.tile([128, NSEG * ZD], bf16)
            zl3 = zln[:].rearrange("p (s d) -> p s d", d=ZD)
            for h in range(2):
                hs = slice(h * BC, (h + 1) * BC)
                nc.vector.reduce_sum(out=s1[:, hs], in_=z3[:, hs, :], axis=AX.X)
                nc.scalar.activation(
                    sq[:, h * BC * ZD : (h + 1) * BC * ZD],
                    zall[:, h * BC * ZD : (h + 1) * BC * ZD],
                    AF.Square,
                )
                nc.vector.reduce_sum(out=s2[:, hs], in_=sq3[:, hs, :], axis=AX.X)
                nc.vector.tensor_scalar_mul(mu[:, hs], s1[:, hs], 1.0 / ZD)
                nc.vector.tensor_mul(musq[:, hs], mu[:, hs], mu[:, hs])
                # vv = var + eps = s2/ZD + eps - mu^2 (fused)
                nc.vector.tensor_scalar(
                    out=vv[:, hs], in0=s2[:, hs], scalar1=1.0 / ZD,
                    scalar2=LN_EPS, op0=ALU.mult, op1=ALU.add,
                )
                nc.vector.tensor_tensor(
                    out=vv[:, hs], in0=vv[:, hs], in1=musq[:, hs],
                    op=ALU.subtract,
                )
                nc.vector.tensor_scalar(
                    out=y[:, hs].bitcast(i32), in0=vv[:, hs].bitcast(i32),
                    scalar1=1, scalar2=None, op0=ALU.arith_shift_right,
                )
                nc.vector.tensor_scalar(
                    out=y[:, hs].bitcast(i32), in0=y[:, hs].bitcast(i32),
                    scalar1=-1, scalar2=0x5F3759DF, op0=ALU.mult, op1=ALU.add,
                )
                for _ in range(2):
                    nc.vector.tensor_mul(t1[:, hs], y[:, hs], y[:, hs])
                    nc.vector.tensor_mul(t1[:, hs], t1[:, hs], vv[:, hs])
                    nc.vector.tensor_scalar(
                        out=t1[:, hs], in0=t1[:, hs], scalar1=-0.5, scalar2=1.5,
                        op0=ALU.mult, op1=ALU.add,
                    )
                    nc.vector.tensor_mul(y[:, hs], y[:, hs], t1[:, hs])
                rstd = y
                # zero out masked negative rows (odd segments of this half)
                nc.vector.tensor_mul(
                    rstd[:, h * BC + 1 : (h + 1) * BC : 2],
                    rstd[:, h * BC + 1 : (h + 1) * BC : 2],
                    keep[:, h * (BC // 2) : (h + 1) * (BC // 2)],
                )
                # normalize this half's batch pairs
                for ch in range(h * (BC // 2), (h + 1) * (BC // 2)):
                    ssl = slice(2 * ch, 2 * ch + 2)
                    mu_bc = mu[:, ssl].unsqueeze(-1).to_broadcast([128, 2, ZD])
                    rstd_bc = rstd[:, ssl].unsqueeze(-1).to_broadcast(
                        [128, 2, ZD]
                    )
                    nc.vector.tensor_tensor(
                        out=zl3[:, ssl, :], in0=z3[:, ssl, :], in1=mu_bc,
                        op=ALU.subtract,
                    )
                    nc.vector.tensor_tensor(
                        out=zl3[:, ssl, :], in0=zl3[:, ssl, :], in1=rstd_bc,
                        op=ALU.mult,
                    )

            # ---- per-batch: transposes, MT matmul, exp/den; diag pre-exp ----
            outv = singles.tile([128, NSEG], f32)  # [num | den]
            diagm = singles.tile([128, BC], f32)
            for p in range(BC // 2):
                pzt = ppzt.tile([128, 512], bf16, tag="pzt")
                for i in range(4):
                    s = 4 * p + i
                    nc.tensor.transpose(
                        out=pzt[:, i * 128 : (i + 1) * 128],
                        in_=zln[:, s * ZD : (s + 1) * ZD],
                        identity=ident_b,
                    )
                zt_sb = scratch.tile([128, 512], bf16, tag="zt")
                if p % 2 == 0:
                    nc.vector.tensor_copy(zt_sb[:], pzt[:])
                else:
                    nc.scalar.copy(zt_sb[:], pzt[:])
                for k in range(2):
                    b = 2 * p + k
                    pmt_b = ppmt.tile([128, 256], f32, tag="pmt")
                    nc.tensor.matmul(
                        out=pmt_b[:],
                        lhsT=e_sb[:, b * L : (b + 1) * L],
                        rhs=zt_sb[:, k * 256 : (k + 1) * 256],
                        start=True,
                        stop=True,
                    )
                    numt = scratch.tile([128, 128], f32, tag="numt")
                    nc.vector.tensor_mul(numt[:], pmt_b[:, 0:128], identf[:])
                    nc.vector.reduce_sum(
                        out=diagm[:, b : b + 1], in_=numt[:], axis=AX.X
                    )
                    expt = scratch.tile([128, 256], f32, tag="expt")
                    nc.scalar.activation(
                        expt[:], pmt_b[:], AF.Exp,
                        accum_out=outv[:, BC + b : BC + b + 1],
                    )
            nc.scalar.activation(outv[:, 0:BC], diagm[:], AF.Exp)
            nc.sync.dma_start(out_d.ap(), outv[:])

    nc.compile()
    return nc


def _prep_in_maps(z, c, mask, neg_map, W, ln_weight):
    import ml_dtypes

    bf = ml_dtypes.bfloat16
    z = np.asarray(z, dtype=np.float32)
    c = np.asarray(c, dtype=np.float32)
    mask = np.asarray(mask).astype(np.int64)
    neg_map = np.asarray(neg_map).astype(np.int64)
    W = np.asarray(W, dtype=np.float32)
    ln_weight = np.asarray(ln_weight, dtype=np.float32)

    wt = (ln_weight[:, None] * W).T  # [c, z] = W'[z, c]
    wpack = np.ascontiguousarray(
        np.concatenate([wt, np.eye(128, dtype=np.float32)], axis=1)
    ).astype(bf)
    boff = np.arange(BC, dtype=np.int64)[None, :]
    in_maps = []
    for i in range(NCORES):
        bsl = slice(i * BC, (i + 1) * BC)
        zf = np.ascontiguousarray(z[:, bsl, :]).reshape(SEQ * BC, ZD).astype(bf)
        cf = np.ascontiguousarray(c[:, bsl, :]).reshape(SEQ * BC, CD).astype(bf)
        m = mask[:, bsl]
        n = neg_map[:, bsl]
        pos_idx = (m * BC + boff).astype(np.int32)
        neg_idx = (n * BC + boff).astype(np.int32)
        zidx = np.empty((L, NSEG), dtype=np.int32)
        zidx[:, 0::2] = pos_idx
        zidx[:, 1::2] = neg_idx
        hit = (n[:, None, :] == m[None, :, :]).any(axis=1)  # [L, BC]
        keep = (~hit).astype(np.float32)
        meta = np.concatenate(
            [zidx, keep.view(np.int32), pos_idx], axis=1
        ).astype(np.int32)
        in_maps.append({"zf": zf, "cf": cf, "meta": meta, "wpack": wpack})
    return in_maps


def _combine(results):
    total = np.float64(0.0)
    for r in results:
        o = np.asarray(r["out"], dtype=np.float64)
        num, den = o[:, 0:BC], o[:, BC : 2 * BC]
        total += np.log(num / den + 1e-3).sum()
    return np.float32(-(total / (L * B)))


def kernel(z, c, mask, neg_map, W, ln_weight, ln_bias):
    from concourse import bass_utils

    global _cached
    if _cached is None:
        _cached = _build_program()
    nc = _cached

    in_maps = _prep_in_maps(z, c, mask, neg_map, W, ln_weight)
    res = bass_utils.run_bass_kernel_spmd(
        nc, in_maps, core_ids=list(range(NCORES))
    )
    return _combine(res.results)

